# revision 1
# baseline (speedup 1.0000x reference)
"""Trainium2 Bass kernel: MultiHeadCrossAttentionWithBias.

Reference computation (per batch b):
  q_u = scale*(u_enc @ wq + wq_b); k/v from e_enc (and vice versa)
  ue_w = softmax(q_u k_e^T + bpp + mask*-inf); u_ctx = ue_w @ v_e
  u_update = u_ctx @ wo + wo_b                     (same mirrored for e)

Sharding: the problem decomposes into 8 fully independent attention units:
(batch b, direction d) for b in 0..3, d in {u->e, e->u}. Core i = (d, b)
handles one unit end-to-end; no collectives needed.

Host prep is layout/precision only (transposes, slices, fp32->fp32r
rounding of matmul operands); all FLOPs run on device.

Per-core inputs:
  encQT  [D=512, L=1024] f32r  query-side encoder, transposed
  encKT  [D=512, L=1024] f32r  key-side encoder, transposed
  bpp    [L, L] f32            logit bias oriented [k, q]
  mask   [L, L] uint8          mask oriented [k, q]
  wq/wk/wv [D, 512] f32r, wo [512, D] f32r, biases f32

On-device math (per core):
  qT[f, s] = scale*(wq^T encQT + wq_b)   (f = h*64+hd on partitions)
  kT[f, s] =        wk^T encKT + wk_b
  v[s, f]  =        encKT^T wv + wv_b    (+ fused ones column per head)
  CB[k, q] = bpp_w*bpp + bpp_b + (mask-1)*1e30      (gpsimd)
  per head h, k-chunk kc:
      S^T[k,q] = kT_h^T qT_h       (PE, PSUM)
      S^T += CB[kc]                (DVE scalar_tensor_tensor in-place)
      E = exp(S^T)                 (ACT; no max-subtraction: logits O(10))
      [ctx^T; den] += [v_h | 1]^T E  (PE, PSUM accumulation over kc)
  rcp = approx_reciprocal(den); broadcast via DRAM bounce
  ctxn[pair] = ctx^T * rcp  (DVE, odd head written to partitions 64..127)
  out[s, e] = sum_pair ctxn_p^T wo_p + wo_b   (PE + DVE bias-add eviction)
"""

import numpy as np
from contextlib import ExitStack

import concourse.bass as bass
import concourse.tile as tile
import concourse.bacc as bacc
import concourse.mybir as mybir
from concourse.masks import make_identity
from concourse import bass_utils

F32 = mybir.dt.float32
F32R = mybir.dt.float32r
U8 = mybir.dt.uint8
BF16 = mybir.dt.bfloat16
AF = mybir.ActivationFunctionType
ALU = mybir.AluOpType

B, L, D, H, HD = 4, 1024, 512, 8, 64
P = 128
FH = H * HD            # 512
SCALE = 1.0 / np.sqrt(HD)
NEG = -1.0e30
N_CORES = 8

# switches
PV_BF16 = True          # E/v in bf16 for the PV matmul (2x PE rate, ~1.3e-3 err)


def bcast_ap(dram_ap, parts):
    """Partition-step-0 broadcast AP over a DRAM row."""
    return bass.AP(tensor=dram_ap.tensor, offset=dram_ap.offset,
                   ap=[[0, parts]] + list(dram_ap.ap))


def round_f32r(x):
    """Round fp32 -> fp32r (11-bit mantissa, RNE), keeping fp32 layout."""
    u = np.ascontiguousarray(x, np.float32).view(np.uint32)
    r = (u + 0x7FF + ((u >> 12) & 1)) & np.uint32(0xFFFFF000)
    return r.view(np.float32)


def build_module():
    nc = bacc.Bacc("TRN2", target_bir_lowering=False, debug=False)

    encQT_d = nc.dram_tensor("encQT", [D, L], F32R, kind="ExternalInput")
    encKT_d = nc.dram_tensor("encKT", [D, L], F32R, kind="ExternalInput")
    wq_d = nc.dram_tensor("wq", [D, FH], F32R, kind="ExternalInput")
    wk_d = nc.dram_tensor("wk", [D, FH], F32R, kind="ExternalInput")
    wv_d = nc.dram_tensor("wv", [D, FH], F32R, kind="ExternalInput")
    wo_d = nc.dram_tensor("wo", [FH, D], F32R, kind="ExternalInput")
    bpp_d = nc.dram_tensor("bpp", [L, L], F32, kind="ExternalInput")
    mask_d = nc.dram_tensor("mask", [L, L], U8, kind="ExternalInput")
    wqb_d = nc.dram_tensor("wqb", [FH], F32, kind="ExternalInput")
    wkb_d = nc.dram_tensor("wkb", [FH], F32, kind="ExternalInput")
    wvb_d = nc.dram_tensor("wvb", [FH], F32, kind="ExternalInput")
    wob_d = nc.dram_tensor("wob", [D], F32, kind="ExternalInput")
    bppw_d = nc.dram_tensor("bppw", [1, 1], F32, kind="ExternalInput")
    bppb_d = nc.dram_tensor("bppb", [1, 1], F32, kind="ExternalInput")
    out_d = nc.dram_tensor("out", [L, D], F32, kind="ExternalOutput")
    den_d = nc.dram_tensor("den_scratch", [H, L], F32, kind="Internal")

    with tile.TileContext(nc) as tc, ExitStack() as ctx:
        const = ctx.enter_context(tc.tile_pool(name="const", bufs=1))
        qkT_p = ctx.enter_context(tc.tile_pool(name="qkT", bufs=8))
        v_p = ctx.enter_context(tc.tile_pool(name="v", bufs=8))
        wo_p = ctx.enter_context(tc.tile_pool(name="wo", bufs=4))
        cb_p = ctx.enter_context(tc.tile_pool(name="cb", bufs=8))
        ps_s = tc.alloc_tile_pool(name="ps_s", bufs=3, space="PSUM")
        ps_c = tc.alloc_tile_pool(name="ps_c", bufs=2, space="PSUM")

        # ---- small bias prep (tiny DMAs) ----
        # bpp_w / bpp_b broadcast to [128,1] columns
        bw_col = const.tile([P, 1], F32)
        nc.gpsimd.dma_start(bw_col[:], bcast_ap(bppw_d.ap()[0:1, :], P))
        bb_col = const.tile([P, 1], F32)
        nc.gpsimd.dma_start(bb_col[:], bcast_ap(bppb_d.ap()[0:1, :], P))
        # projection biases
        wqb_raw = const.tile([P, 4], F32)
        nc.gpsimd.dma_start(wqb_raw[:], wqb_d.ap().rearrange("(c p) -> p c", p=P))
        wqb_sc = const.tile([P, 4], F32)
        nc.vector.tensor_scalar_mul(wqb_sc[:], wqb_raw[:], float(SCALE))
        wkb_c = const.tile([P, 4], F32)
        nc.gpsimd.dma_start(wkb_c[:], wkb_d.ap().rearrange("(c p) -> p c", p=P))
        wvb_bc = const.tile([P, FH], F32)
        nc.gpsimd.dma_start(wvb_bc[:], bcast_ap(wvb_d.ap(), P))
        wob_bc = const.tile([P, D], F32)
        nc.gpsimd.dma_start(wob_bc[:], bcast_ap(wob_d.ap(), P))


        # ---- projections ----
        cb = []
        qT, kT, v_aug = [], [], []
        with tc.tile_pool(name="enc", bufs=8) as enc_p, \
             tc.tile_pool(name="wqkv", bufs=12) as w_p:
            eq, ek = [], []
            wq_t, wk_t, wv_t = [], [], []
            # load in first-use order: wq -> encQT -> wk -> encKT -> wv
            for w_dram, wlst, elst, edram in (
                (wq_d, wq_t, eq, encQT_d), (wk_d, wk_t, ek, encKT_d),
                (wv_d, wv_t, None, None),
            ):
                for dc in range(4):
                    t = w_p.tile([P, FH], F32R, tag="w",
                                 name=f"w_{w_dram.name}{dc}")
                    nc.sync.dma_start(t[:], w_dram.ap()[dc * P:(dc + 1) * P, :])
                    wlst.append(t)
                if elst is None:
                    continue
                for dc in range(4):
                    t = enc_p.tile([P, L], F32R, tag="enc",
                                   name=f"enc_{edram.name}{dc}")
                    nc.sync.dma_start(t[:], edram.ap()[dc * P:(dc + 1) * P, :])
                    elst.append(t)


            # ---- combined bias CB[k, q] = (bpp*w + b) + (mask-1)*1e30 ----
            # emitted after projection-critical loads so its DMA traffic
            # (5MB) does not delay the first matmuls; compute overlaps
            # projections. (m*1e30) + (-1e30) is exact for m in {0,1}.
            negbig = const.tile([P, 1], F32)
            nc.vector.memset(negbig[:], NEG)
            cbt_p = tc.alloc_tile_pool(name="cbtmp", bufs=3)
            for kc in range(8):
                m_t = cbt_p.tile([P, L], U8, tag="m", name=f"m{kc}")
                nc.sync.dma_start(m_t[:], mask_d.ap()[kc * P:(kc + 1) * P, :])
                mn_t = cbt_p.tile([P, L], F32, tag="mn", name=f"mn{kc}")
                nc.scalar.activation(mn_t[:], m_t[:], AF.Identity,
                                     bias=negbig[:], scale=-NEG)
                b_t = cbt_p.tile([P, L], F32, tag="b", name=f"b{kc}")
                nc.sync.dma_start(b_t[:], bpp_d.ap()[kc * P:(kc + 1) * P, :])
                bs_t = cbt_p.tile([P, L], F32, tag="bs", name=f"bs{kc}")
                nc.vector.tensor_scalar(bs_t[:], b_t[:], bw_col[:, 0:1],
                                        bb_col[:, 0:1], ALU.mult, ALU.add)
                c_t = cb_p.tile([P, L], F32, tag="cb", name=f"cb{kc}")
                nc.gpsimd.tensor_add(c_t[:], mn_t[:], bs_t[:])
                cb.append(c_t)
            cbt_p.release()

            # qT / kT: [f, s] packed two heads per 128-partition chunk
            for which, w_t, enc_t, out_list in (
                ("q", wq_t, eq, qT), ("k", wk_t, ek, kT),
            ):
                for pc in range(4):
                    o = qkT_p.tile([P, L], F32R, tag="qkT",
                                   name=f"{which}T{pc}")
                    for sh in range(2):
                        ps = ps_s.tile([P, 512], F32, tag="ps_s",
                                       name=f"ps_{which}{pc}_{sh}")
                        for dc in range(4):
                            nc.tensor.matmul(
                                ps[:],
                                w_t[dc][:, pc * P:(pc + 1) * P],
                                enc_t[dc][:, sh * 512:(sh + 1) * 512],
                                start=(dc == 0), stop=(dc == 3))
                        sl = slice(sh * 512, (sh + 1) * 512)
                        if which == "q":
                            nc.scalar.activation(o[:, sl], ps[:], AF.Identity,
                                                 bias=wqb_sc[:, pc:pc + 1],
                                                 scale=float(SCALE))
                        else:
                            nc.scalar.activation(o[:, sl], ps[:], AF.Identity,
                                                 bias=wkb_c[:, pc:pc + 1],
                                                 scale=1.0)
                    out_list.append(o)

            # v: [s, f] with ones column interleaved per head ([128, 8*65])
            for sc in range(8):
                ps = ps_s.tile([P, 512], F32, tag="ps_s", name=f"ps_v{sc}")
                for dc in range(4):
                    nc.tensor.matmul(ps[:], ek[dc][:, sc * P:(sc + 1) * P],
                                     wv_t[dc][:], start=(dc == 0),
                                     stop=(dc == 3))
                va = v_p.tile([P, H * (HD + 1)], BF16 if PV_BF16 else F32R, tag="v", name=f"v{sc}")
                vg = va[:].rearrange("p (h c) -> p h c", c=HD + 1)
                nc.vector.scalar_tensor_tensor(
                    vg[:, :, 0:HD],
                    ps[:].rearrange("p (h c) -> p h c", c=HD), 1.0,
                    wvb_bc[:].rearrange("p (h c) -> p h c", c=HD),
                    ALU.bypass, ALU.add)
                ones_ap = vg[:, :, HD:HD + 1]
                if not PV_BF16:
                    ones_ap = ones_ap.bitcast(F32)
                nc.vector.memset(ones_ap, 1.0)
                v_aug.append(va)

        # ---- wo loads (late: not projection-critical) ----
        wo_t = []
        for p_ in range(4):
            t = wo_p.tile([P, D], F32R, tag="wo", name=f"wo{p_}")
            nc.sync.dma_start(t[:], wo_d.ap()[p_ * P:(p_ + 1) * P, :])
            wo_t.append(t)

        # ---- attention ----
        # Per (head, kc): QK (PE, fp32r) -> +CB (DVE, in-place on PSUM) ->
        # exp (ACT, -> bf16) -> PV (PE, bf16). PV is emitted with a lag of
        # 2 kc steps so the in-order PE stream never waits on exp.
        ctxn_p = ctx.enter_context(tc.tile_pool(name="ctxn", bufs=4))
        ctxr_p = ctx.enter_context(tc.tile_pool(name="ctxr", bufs=2))
        den_p = ctx.enter_context(tc.tile_pool(name="den", bufs=2))
        ctxn = [None] * 4
        with tc.tile_pool(name="e", bufs=8) as e_p, \
             tc.tile_pool(name="rb", bufs=2) as rb_p:
            for h in range(H):
                LAG = 4 if h == 0 else 3
                o = (h % 2) * HD
                pc = h // 2
                c_ps = [ps_c.tile([HD + 1, 512], F32, tag="ps_c",
                                  name=f"c_ps_{h}_{i}")
                        for i in range(2)]
                e_ts = {}
                for kc in range(8 + LAG):
                    if kc < 8:
                        s_ps = ps_s.tile([P, L], F32, tag="ps_s",
                                         name=f"s_ps_{h}_{kc}")
                        for qh in range(2):
                            sl = slice(qh * 512, (qh + 1) * 512)
                            nc.tensor.matmul(
                                s_ps[:, sl],
                                kT[pc][o:o + HD, kc * P:(kc + 1) * P],
                                qT[pc][o:o + HD, sl],
                                start=True, stop=True)
                        nc.vector.scalar_tensor_tensor(
                            s_ps[:], s_ps[:], 1.0, cb[kc][:],
                            ALU.bypass, ALU.add)
                        et = e_p.tile([P, L], BF16 if PV_BF16 else F32R,
                                      tag="e", name=f"e_{h}_{kc}")
                        nc.scalar.activation(et[:], s_ps[:], AF.Exp)
                        e_ts[kc] = et
                    if kc >= LAG:
                        kp = kc - LAG
                        for qh in range(2):
                            sl = slice(qh * 512, (qh + 1) * 512)
                            nc.tensor.matmul(
                                c_ps[qh][:],
                                v_aug[kp][:, h * (HD + 1):(h + 1) * (HD + 1)],
                                e_ts[kp][:, sl],
                                start=(kp == 0), stop=(kp == 7))
                # evict raw ctx + denominator rows (ACT, partition-shifted),
                # freeing PSUM; normalize later from SBUF.
                if h % 2 == 0:
                    ctxn[pc] = ctxn_p.tile([P, L], F32R, tag="ctxn",
                                           name=f"ctxn{pc}")
                    ctxr = ctxr_p.tile([P, L], F32, tag="ctxr",
                                       name=f"ctxr{pc}")
                    den_sb = den_p.tile([97, 512], F32, tag="den",
                                        name=f"den{pc}")
                    nc.vector.memset(den_sb[:], 1.0)
                    ctxr_hold = (ctxr, den_sb)
                else:
                    ctxr, den_sb = ctxr_hold
                for qh in range(2):
                    sl = slice(qh * 512, (qh + 1) * 512)
                    nc.scalar.copy(ctxr[o:o + HD, sl], c_ps[qh][0:HD, :])
                    r_ = ((h % 2) * 2 + qh) * 32
                    nc.scalar.copy(den_sb[r_:r_ + 1, :],
                                   c_ps[qh][HD:HD + 1, :])
                if h % 2 == 1:
                    # batched reciprocal for the pair (4 rows x 512)
                    rcp = den_p.tile([97, 512], F32, tag="rcp",
                                     name=f"rcp{pc}")
                    nc.vector.reciprocal(rcp[:], den_sb[:])
                    dflat = den_d.ap()[2 * pc:2 * pc + 2, :].rearrange(
                        "h (a b) -> (h a) b", b=512)
                    for r_ in range(4):
                        nc.sync.dma_start(dflat[r_:r_ + 1, :],
                                          rcp[32 * r_:32 * r_ + 1, :])
                    rb = rb_p.tile([P, L], F32, tag="rb", name=f"rb{pc}")
                    nc.gpsimd.dma_start(
                        rb[0:HD, :], bcast_ap(den_d.ap()[2 * pc:2 * pc + 1, :], HD))
                    nc.gpsimd.dma_start(
                        rb[HD:P, :], bcast_ap(den_d.ap()[2 * pc + 1:2 * pc + 2, :], HD))
                    nc.vector.scalar_tensor_tensor(
                        ctxn[pc][:], ctxr[:], 1.0, rb[:],
                        ALU.bypass, ALU.mult)

        # ---- output projection ----
        # p-major emission: all pair-0 matmuls first, so the PE only waits
        # on the last pair's normalize chain for the final 8 matmuls.
        ps_c.release()
        ps_s.release()
        ps_o = tc.alloc_tile_pool(name="ps_o", bufs=8, space="PSUM")
        with tc.tile_pool(name="outp", bufs=3) as out_p:
            o_ps = [ps_o.tile([P, D], F32, tag="ps_o", name=f"o_ps{st}")
                    for st in range(8)]
            for p_ in range(4):
                for st in range(8):
                    nc.tensor.matmul(o_ps[st][:],
                                     ctxn[p_][:, st * P:(st + 1) * P],
                                     wo_t[p_][:],
                                     start=(p_ == 0), stop=(p_ == 3))
            for st in range(8):
                o_t = out_p.tile([P, D], F32, tag="out", name=f"out{st}")
                nc.vector.scalar_tensor_tensor(
                    o_t[:], o_ps[st][:], 1.0, wob_bc[:], ALU.bypass, ALU.add)
                nc.sync.dma_start(out_d.ap()[st * P:(st + 1) * P, :], o_t[:])
        ps_o.release()

    nc.compile()
    return nc


def shard_inputs(u_enc, e_enc, logit_bpp, ue_mask, eu_mask,
                 wq_k, wq_b, wk_k, wk_b, wv_k, wv_b, wo_k, wo_b,
                 bpp_w, bpp_b):
    """Build the 8 per-core input maps (layout + fp32r rounding only)."""
    u_enc = np.asarray(u_enc, np.float32)
    e_enc = np.asarray(e_enc, np.float32)
    bpp = np.asarray(logit_bpp, np.float32)
    ue_m = np.asarray(ue_mask).astype(np.uint8)
    eu_m = np.asarray(eu_mask).astype(np.uint8)
    com = dict(
        wq=round_f32r(np.asarray(wq_k, np.float32).reshape(D, FH)),
        wk=round_f32r(np.asarray(wk_k, np.float32).reshape(D, FH)),
        wv=round_f32r(np.asarray(wv_k, np.float32).reshape(D, FH)),
        wo=round_f32r(np.asarray(wo_k, np.float32).reshape(FH, D)),
        wqb=np.asarray(wq_b, np.float32).reshape(FH).copy(),
        wkb=np.asarray(wk_b, np.float32).reshape(FH).copy(),
        wvb=np.asarray(wv_b, np.float32).reshape(FH).copy(),
        wob=np.asarray(wo_b, np.float32).reshape(D).copy(),
        bppw=np.asarray(bpp_w, np.float32).reshape(1, 1).copy(),
        bppb=np.asarray(bpp_b, np.float32).reshape(1, 1).copy(),
    )
    uT = [round_f32r(u_enc[b].T) for b in range(B)]
    eT = [round_f32r(e_enc[b].T) for b in range(B)]
    bppT = np.ascontiguousarray(bpp.T)
    in_maps = []
    for i in range(N_CORES):
        d, b = divmod(i, B)
        if d == 0:      # u queries, e keys -> u_update[b]
            m = dict(encQT=uT[b], encKT=eT[b], bpp=bppT,
                     mask=np.ascontiguousarray(ue_m[b, 0].T))
        else:           # e queries, u keys -> e_update[b]
            m = dict(encQT=eT[b], encKT=uT[b], bpp=bpp,
                     mask=np.ascontiguousarray(eu_m[b, 0].T))
        m.update(com)
        in_maps.append(m)
    return in_maps


_NC = None


def kernel(**inputs):
    global _NC
    if _NC is None:
        _NC = build_module()
    in_maps = shard_inputs(**inputs)
    res = bass_utils.run_bass_kernel_spmd(
        _NC, in_maps, core_ids=list(range(N_CORES)))
    u_update = np.stack([res.results[b]["out"] for b in range(B)])
    e_update = np.stack([res.results[B + b]["out"] for b in range(B)])
    return u_update, e_update


if __name__ == "__main__":
    # single-core CoreSim check of one (direction, batch) unit
    from concourse.bass_interp import CoreSim

    rng = np.random.default_rng(0)
    u = rng.standard_normal((B, L, D)).astype(np.float32)
    e = rng.standard_normal((B, L, D)).astype(np.float32)
    bpp = rng.standard_normal((L, L)).astype(np.float32)
    uem = (rng.random((B, 1, L, L)) < 0.9)
    eum = (rng.random((B, 1, L, L)) < 0.9)
    w = 1.0 / np.sqrt(D)
    wq = (rng.standard_normal((D, H, HD)) * w).astype(np.float32)
    wk = (rng.standard_normal((D, H, HD)) * w).astype(np.float32)
    wv = (rng.standard_normal((D, H, HD)) * w).astype(np.float32)
    wo = (rng.standard_normal((H, HD, D)) / np.sqrt(FH)).astype(np.float32)
    zq = (rng.standard_normal((H, HD)) * 0.1).astype(np.float32)
    zo = (rng.standard_normal((D,)) * 0.1).astype(np.float32)

    nc = build_module()
    in_maps = shard_inputs(u, e, bpp, uem, eum, wq, zq, wk, zq, wv, zq,
                           wo, zo, np.float32(1.3), np.float32(-0.2))

    core = 0
    sim = CoreSim(nc, trace=False)
    for k, vv in in_maps[core].items():
        sim.tensor(k)[:] = vv
    sim.simulate(check_with_hw=False)
    got = np.array(sim.tensor("out"))

    def ref_unit(encQ, encK, bias_qk, mask_qk):
        q = SCALE * (encQ @ wq.reshape(D, FH) + zq.reshape(FH))
        kk = encK @ wk.reshape(D, FH) + zq.reshape(FH)
        vv = encK @ wv.reshape(D, FH) + zq.reshape(FH)
        accum = np.zeros((L, D), np.float64)
        for h in range(H):
            qi = q[:, h * HD:(h + 1) * HD]
            ki = kk[:, h * HD:(h + 1) * HD]
            vi = vv[:, h * HD:(h + 1) * HD]
            s = qi @ ki.T + bias_qk
            s = np.where(mask_qk, s, -np.inf)
            s = s - s.max(-1, keepdims=True)
            p_ = np.exp(s)
            p_ /= p_.sum(-1, keepdims=True)
            accum += (p_ @ vi) @ wo[h]
        return (accum + zo).astype(np.float32)

    bq = 1.3 * bpp + -0.2
    exp_out = ref_unit(u[0], e[0], bq, uem[0, 0])
    err = np.abs(got - exp_out).max() / np.abs(exp_out).max()
    print("unit relerr vs numpy:", err)



# revision 5
# speedup vs baseline: 1.1686x; 1.1686x over previous
"""Trainium2 Bass kernel: MultiHeadCrossAttentionWithBias.

Reference computation (per batch b):
  q_u = scale*(u_enc @ wq + wq_b); k/v from e_enc (and vice versa)
  ue_w = softmax(q_u k_e^T + bppw*bpp + bppb + mask*-inf); u_ctx = ue_w @ v_e
  u_update = u_ctx @ wo + wo_b                     (same mirrored for e)

Sharding: 8 fully independent attention units (batch b, direction d).
Core i = (d, b) handles one unit end-to-end; no collectives.

v2 design notes (vs the fp32r baseline at 219us):
 - ALL matmul operands bf16: fp32r ran in fp32_mode=HIGH at ~630ns per
   N=512 matmul and kept the PE HAM-throttled at 1.2GHz; bf16 runs at
   ~216ns warm. PE work drops ~227us -> ~83us.
 - The per-(h,kc) DVE bias-add into PSUM (94us) is replaced by a bf16
   multiply with a precomputed EBM[k,q] = exp(bppw*bpp + maskneg):
   exp(S+CB) = exp(S)*EBM. bppb and the reference's +EPS shift all
   logits uniformly and cancel in softmax, so they are dropped.
   Mask lands as exp(-1e30) = 0, which also implements the
   post-softmax re-mask for free.
 - ACT does exp only (+ tiny den evictions); ctx eviction + normalize
   on DVE (partition-shifted writes verified in CoreSim).
 - reciprocal_approx_fast (1 DVE pass, ~4e-6 rel err) instead of the
   8-pass iterative reciprocal.
 - den reciprocal broadcast along partitions via DRAM bounce.

Host prep is layout/precision only (transposes, slices, bf16 rounding,
mask -> {0,-1e30} encoding); all FLOPs run on device.
"""

import numpy as np
from contextlib import ExitStack

import ml_dtypes

import concourse.bass as bass
import concourse.tile as tile
import concourse.bacc as bacc
import concourse.mybir as mybir
from concourse import bass_utils

F32 = mybir.dt.float32
F16 = mybir.dt.float16
U8 = mybir.dt.uint8
BF16 = mybir.dt.bfloat16
AF = mybir.ActivationFunctionType
ALU = mybir.AluOpType
BF16NP = ml_dtypes.bfloat16

B, L, D, H, HD = 4, 1024, 512, 8, 64
P = 128
FH = H * HD            # 512
SCALE = 1.0 / np.sqrt(HD)
NEG = -1.0e30
N_CORES = 8
LAGS = (4, 3)          # kc lag between QK and PV streams (h==0, h>0)


def bcast_ap(dram_ap, parts):
    """Partition-step-0 broadcast AP over a DRAM row."""
    return bass.AP(tensor=dram_ap.tensor, offset=dram_ap.offset,
                   ap=[[0, parts]] + list(dram_ap.ap))


def build_module():
    nc = bacc.Bacc("TRN2", target_bir_lowering=False, debug=False)

    encQT_d = nc.dram_tensor("encQT", [D, L], F16, kind="ExternalInput")
    encKT_d = nc.dram_tensor("encKT", [D, L], F16, kind="ExternalInput")
    wq_d = nc.dram_tensor("wq", [D, FH], F16, kind="ExternalInput")
    wk_d = nc.dram_tensor("wk", [D, FH], F16, kind="ExternalInput")
    wv_d = nc.dram_tensor("wv", [D, FH], F16, kind="ExternalInput")
    wo_d = nc.dram_tensor("wo", [FH, D], F16, kind="ExternalInput")
    bpp_d = nc.dram_tensor("bpp", [L, L], F16, kind="ExternalInput")
    mneg_d = nc.dram_tensor("mneg", [L, L], F16, kind="ExternalInput")
    wqb_d = nc.dram_tensor("wqb", [FH], F32, kind="ExternalInput")
    wkb_d = nc.dram_tensor("wkb", [FH], F32, kind="ExternalInput")
    wvb_d = nc.dram_tensor("wvb", [FH], F32, kind="ExternalInput")
    wob_d = nc.dram_tensor("wob", [D], F32, kind="ExternalInput")
    bppw_d = nc.dram_tensor("bppw", [1, 1], F32, kind="ExternalInput")
    out_d = nc.dram_tensor("out", [L, D], F32, kind="ExternalOutput")
    den_d = nc.dram_tensor("den_scratch", [H, L], F32, kind="Internal")

    with tile.TileContext(nc) as tc, ExitStack() as ctx:
        const = ctx.enter_context(tc.tile_pool(name="const", bufs=1))
        qkT_p = ctx.enter_context(tc.tile_pool(name="qkT", bufs=8))
        v_p = ctx.enter_context(tc.tile_pool(name="v", bufs=8))
        wo_p = ctx.enter_context(tc.tile_pool(name="wo", bufs=4))
        ebm_p = ctx.enter_context(tc.tile_pool(name="ebm", bufs=8))
        ps_s = tc.alloc_tile_pool(name="ps_s", bufs=2, space="PSUM")
        ps_c = tc.alloc_tile_pool(name="ps_c", bufs=2, space="PSUM")

        # ---- small bias prep (tiny DMAs) ----
        bw_col = const.tile([P, 1], F32)
        nc.gpsimd.dma_start(bw_col[:], bcast_ap(bppw_d.ap()[0:1, :], P))
        wqb_raw = const.tile([P, 4], F32)
        nc.gpsimd.dma_start(wqb_raw[:], wqb_d.ap().rearrange("(c p) -> p c", p=P))
        wqb_sc = const.tile([P, 4], F32)
        nc.vector.tensor_scalar_mul(wqb_sc[:], wqb_raw[:], float(SCALE))
        wkb_c = const.tile([P, 4], F32)
        nc.gpsimd.dma_start(wkb_c[:], wkb_d.ap().rearrange("(c p) -> p c", p=P))
        wvb_bc = const.tile([P, FH], F32)
        nc.gpsimd.dma_start(wvb_bc[:], bcast_ap(wvb_d.ap(), P))
        wob_bc = const.tile([P, D], F32)
        nc.gpsimd.dma_start(wob_bc[:], bcast_ap(wob_d.ap(), P))

        # ---- projections (all-bf16 matmuls) ----
        ebm = []
        qT, kT, v_aug = [], [], []
        with tc.tile_pool(name="enc", bufs=8) as enc_p, \
             tc.tile_pool(name="wqkv", bufs=12) as w_p:
            eq, ek = [], []
            wq_t, wk_t, wv_t = [], [], []
            # load in first-use order: wq -> encQT -> wk -> encKT -> wv
            for w_dram, wlst, elst, edram in (
                (wq_d, wq_t, eq, encQT_d), (wk_d, wk_t, ek, encKT_d),
                (wv_d, wv_t, None, None),
            ):
                for dc in range(4):
                    t = w_p.tile([P, FH], F16, tag="w",
                                 name=f"w_{w_dram.name}{dc}")
                    nc.sync.dma_start(t[:], w_dram.ap()[dc * P:(dc + 1) * P, :])
                    wlst.append(t)
                if elst is None:
                    continue
                for dc in range(4):
                    t = enc_p.tile([P, L], F16, tag="enc",
                                   name=f"enc_{edram.name}{dc}")
                    nc.sync.dma_start(t[:], edram.ap()[dc * P:(dc + 1) * P, :])
                    elst.append(t)

            # ---- EBM[k, q] = exp(bppw*bpp + mneg), bf16, mask folded in ----
            # emitted after projection-critical loads; compute overlaps
            # the projection matmuls.
            cbt_p = tc.alloc_tile_pool(name="cbtmp", bufs=4)
            for kc in range(8):
                b_t = cbt_p.tile([P, L], F16, tag="b", name=f"b{kc}")
                nc.gpsimd.dma_start(b_t[:], bpp_d.ap()[kc * P:(kc + 1) * P, :])
                m_t = cbt_p.tile([P, L], F16, tag="m", name=f"m{kc}")
                nc.gpsimd.dma_start(m_t[:], mneg_d.ap()[kc * P:(kc + 1) * P, :])
                cb_t = cbt_p.tile([P, L], F32, tag="cb", name=f"cb{kc}")
                nc.vector.scalar_tensor_tensor(
                    cb_t[:], b_t[:], bw_col[:, 0:1], m_t[:],
                    ALU.mult, ALU.add)
                e_t = ebm_p.tile([P, L], BF16, tag="ebm", name=f"ebm{kc}")
                nc.scalar.activation(e_t[:], cb_t[:], AF.Exp)
                ebm.append(e_t)
            cbt_p.release()

            # qT / kT: [f, s] packed two heads per 128-partition chunk
            for which, w_t, enc_t, out_list in (
                ("q", wq_t, eq, qT), ("k", wk_t, ek, kT),
            ):
                for pc in range(4):
                    o = qkT_p.tile([P, L], F16, tag="qkT",
                                   name=f"{which}T{pc}")
                    for sh in range(2):
                        ps = ps_s.tile([P, 512], F32, tag="ps_s",
                                       name=f"ps_{which}{pc}_{sh}")
                        for dc in range(4):
                            nc.tensor.matmul(
                                ps[:],
                                w_t[dc][:, pc * P:(pc + 1) * P],
                                enc_t[dc][:, sh * 512:(sh + 1) * 512],
                                start=(dc == 0), stop=(dc == 3))
                        sl = slice(sh * 512, (sh + 1) * 512)
                        if which == "q":
                            nc.scalar.activation(o[:, sl], ps[:], AF.Identity,
                                                 bias=wqb_sc[:, pc:pc + 1],
                                                 scale=float(SCALE))
                        else:
                            nc.scalar.activation(o[:, sl], ps[:], AF.Identity,
                                                 bias=wkb_c[:, pc:pc + 1],
                                                 scale=1.0)
                    out_list.append(o)

            # v: [s, f] with ones column interleaved per head ([128, 8*65])
            for sc in range(8):
                ps = ps_s.tile([P, 512], F32, tag="ps_s", name=f"ps_v{sc}")
                for dc in range(4):
                    nc.tensor.matmul(ps[:], ek[dc][:, sc * P:(sc + 1) * P],
                                     wv_t[dc][:], start=(dc == 0),
                                     stop=(dc == 3))
                va = v_p.tile([P, H * (HD + 1)], BF16, tag="v", name=f"v{sc}")
                vg = va[:].rearrange("p (h c) -> p h c", c=HD + 1)
                nc.vector.scalar_tensor_tensor(
                    vg[:, :, 0:HD],
                    ps[:].rearrange("p (h c) -> p h c", c=HD), 1.0,
                    wvb_bc[:].rearrange("p (h c) -> p h c", c=HD),
                    ALU.bypass, ALU.add)
                nc.vector.memset(vg[:, :, HD:HD + 1], 1.0)
                v_aug.append(va)

        # ---- wo loads (late: not projection-critical) ----
        wo_t = []
        for p_ in range(4):
            t = wo_p.tile([P, D], F16, tag="wo", name=f"wo{p_}")
            nc.sync.dma_start(t[:], wo_d.ap()[p_ * P:(p_ + 1) * P, :])
            wo_t.append(t)

        # ---- attention ----
        # Per (head, kc): QK (PE, bf16) -> exp (ACT, ->bf16) ->
        # *EBM (DVE, bf16 2x) -> PV (PE, bf16). PV emitted with a lag of
        # LAG kc steps so the in-order PE stream never waits on exp/mult.
        ctxn_p = ctx.enter_context(tc.tile_pool(name="ctxn", bufs=4))
        den_p = ctx.enter_context(tc.tile_pool(name="den", bufs=4))
        ctxn = [None] * 4
        with tc.tile_pool(name="er", bufs=3) as er_p, \
             tc.tile_pool(name="e", bufs=7) as e_p, \
             tc.tile_pool(name="rb", bufs=2) as rb_p:
            for h in range(H):
                LAG = LAGS[0] if h == 0 else LAGS[1]
                o = (h % 2) * HD
                pc = h // 2
                # ctx accumulator: [65, 512] per qh, qh0 in free 0:512,
                # qh1 in 512:1024 (adjacent banks -> 1-op den eviction)
                c_ps = ps_c.tile([P, L], F32, tag="ps_c", name=f"c_ps{h}")
                e_ts = {}
                for kc in range(8 + LAG):
                    if kc < 8:
                        s_ps = ps_s.tile([P, L], F32, tag="ps_s",
                                         name=f"s_ps_{h}_{kc}")
                        for qh in range(2):
                            sl = slice(qh * 512, (qh + 1) * 512)
                            nc.tensor.matmul(
                                s_ps[:, sl],
                                kT[pc][o:o + HD, kc * P:(kc + 1) * P],
                                qT[pc][o:o + HD, sl],
                                start=True, stop=True)
                        er = er_p.tile([P, L], BF16, tag="er",
                                       name=f"er_{h}_{kc}")
                        nc.scalar.activation(er[:], s_ps[:], AF.Exp)
                        et = e_p.tile([P, L], BF16, tag="e",
                                      name=f"e_{h}_{kc}")
                        nc.vector.tensor_mul(et[:], er[:], ebm[kc][:])
                        e_ts[kc] = et
                    if kc >= LAG:
                        kp = kc - LAG
                        for qh in range(2):
                            sl = slice(qh * 512, (qh + 1) * 512)
                            nc.tensor.matmul(
                                c_ps[0:HD + 1, sl],
                                v_aug[kp][:, h * (HD + 1):(h + 1) * (HD + 1)],
                                e_ts[kp][:, sl],
                                start=(kp == 0), stop=(kp == 7))
                # evictions: den row (ACT, partition-shifted) and raw ctx
                # (DVE -> bf16, partition-shifted for odd heads); frees the
                # PSUM banks quickly so PV of h+1 is never blocked.
                if h % 2 == 0:
                    den_sb = den_p.tile([33, L], F32, tag="den",
                                        name=f"den{pc}")
                    nc.vector.memset(den_sb[:], 1.0)
                    ctxn[pc] = ctxn_p.tile([P, L], F16, tag="ctxn",
                                           name=f"ctxn{pc}")
                r0 = (h % 2) * 32
                nc.scalar.copy(den_sb[r0:r0 + 1, :], c_ps[HD:HD + 1, :])
                nc.vector.tensor_copy(ctxn[pc][o:o + HD, :], c_ps[0:HD, :])
                if h % 2 == 1:
                    rcp = den_p.tile([33, L], F32, tag="rcp", name=f"rcp{pc}")
                    nc.vector.reciprocal_approx_fast(rcp[:], den_sb[:])
                    for r_ in range(2):
                        nc.sync.dma_start(den_d.ap()[2 * pc + r_:2 * pc + r_ + 1, :],
                                          rcp[32 * r_:32 * r_ + 1, :])
                    rb = rb_p.tile([P, L], F32, tag="rb", name=f"rb{pc}")
                    nc.gpsimd.dma_start(
                        rb[0:HD, :], bcast_ap(den_d.ap()[2 * pc:2 * pc + 1, :], HD))
                    nc.gpsimd.dma_start(
                        rb[HD:P, :], bcast_ap(den_d.ap()[2 * pc + 1:2 * pc + 2, :], HD))
                    # normalize in place: ctxn *= 1/den
                    nc.vector.tensor_mul(ctxn[pc][:], ctxn[pc][:], rb[:])

        # ---- output projection ----
        # p-major emission: all pair-0 matmuls first, so the PE only waits
        # on the last pair's normalize chain for the final 8 matmuls.
        ps_c.release()
        ps_s.release()
        ps_o = tc.alloc_tile_pool(name="ps_o", bufs=8, space="PSUM")
        with tc.tile_pool(name="outp", bufs=3) as out_p:
            o_ps = [ps_o.tile([P, D], F32, tag="ps_o", name=f"o_ps{st}")
                    for st in range(8)]
            for p_ in range(4):
                for st in range(8):
                    nc.tensor.matmul(o_ps[st][:],
                                     ctxn[p_][:, st * P:(st + 1) * P],
                                     wo_t[p_][:],
                                     start=(p_ == 0), stop=(p_ == 3))
            for st in range(8):
                o_t = out_p.tile([P, D], F32, tag="out", name=f"out{st}")
                nc.vector.scalar_tensor_tensor(
                    o_t[:], o_ps[st][:], 1.0, wob_bc[:], ALU.bypass, ALU.add)
                nc.sync.dma_start(out_d.ap()[st * P:(st + 1) * P, :], o_t[:])
        ps_o.release()

    nc.compile()
    return nc


def shard_inputs(u_enc, e_enc, logit_bpp, ue_mask, eu_mask,
                 wq_k, wq_b, wk_k, wk_b, wv_k, wv_b, wo_k, wo_b,
                 bpp_w, bpp_b):
    """Build the 8 per-core input maps (layout + precision only).

    bpp_b is dropped: it shifts every logit in a row uniformly and
    cancels in softmax (as does the reference's +EPS).
    """
    u_enc = np.asarray(u_enc, np.float32)
    e_enc = np.asarray(e_enc, np.float32)
    bpp = np.asarray(logit_bpp, np.float32)
    ue_m = np.asarray(ue_mask).astype(np.float32)
    eu_m = np.asarray(eu_mask).astype(np.float32)

    def bf(x):
        return np.ascontiguousarray(x).astype(BF16NP)

    def hf(x):
        return np.ascontiguousarray(x).astype(np.float16)

    com = dict(
        wq=hf(np.asarray(wq_k, np.float32).reshape(D, FH)),
        wk=hf(np.asarray(wk_k, np.float32).reshape(D, FH)),
        wv=hf(np.asarray(wv_k, np.float32).reshape(D, FH)),
        wo=hf(np.asarray(wo_k, np.float32).reshape(FH, D)),
        wqb=np.asarray(wq_b, np.float32).reshape(FH).copy(),
        wkb=np.asarray(wk_b, np.float32).reshape(FH).copy(),
        wvb=np.asarray(wv_b, np.float32).reshape(FH).copy(),
        wob=np.asarray(wo_b, np.float32).reshape(D).copy(),
        bppw=np.asarray(bpp_w, np.float32).reshape(1, 1).copy(),
    )
    uT = [hf(u_enc[b].T) for b in range(B)]
    eT = [hf(e_enc[b].T) for b in range(B)]
    bppT = hf(bpp.T)
    bppN = hf(bpp)
    # mask -> additive {0, -1e30} encoding, [k, q] orientation
    ue_neg = [hf((ue_m[b, 0].T - 1.0) * 60000.0) for b in range(B)]
    eu_neg = [hf((eu_m[b, 0].T - 1.0) * 60000.0) for b in range(B)]
    in_maps = []
    for i in range(N_CORES):
        d, b = divmod(i, B)
        if d == 0:      # u queries, e keys -> u_update[b]
            m = dict(encQT=uT[b], encKT=eT[b], bpp=bppT, mneg=ue_neg[b])
        else:           # e queries, u keys -> e_update[b]
            m = dict(encQT=eT[b], encKT=uT[b], bpp=bppN, mneg=eu_neg[b])
        m.update(com)
        in_maps.append(m)
    return in_maps


_NC = None


def kernel(**inputs):
    global _NC
    if _NC is None:
        _NC = build_module()
    in_maps = shard_inputs(**inputs)
    res = bass_utils.run_bass_kernel_spmd(
        _NC, in_maps, core_ids=list(range(N_CORES)))
    u_update = np.stack([res.results[b]["out"] for b in range(B)])
    e_update = np.stack([res.results[B + b]["out"] for b in range(B)])
    return u_update, e_update


if __name__ == "__main__":
    # single-core CoreSim check of one (direction, batch) unit
    from concourse.bass_interp import CoreSim

    rng = np.random.default_rng(0)
    u = rng.standard_normal((B, L, D)).astype(np.float32)
    e = rng.standard_normal((B, L, D)).astype(np.float32)
    bpp = rng.standard_normal((L, L)).astype(np.float32)
    uem = (rng.random((B, 1, L, L)) < 0.9)
    eum = (rng.random((B, 1, L, L)) < 0.9)
    w = 1.0 / np.sqrt(D)
    wq = (rng.standard_normal((D, H, HD)) * w).astype(np.float32)
    wk = (rng.standard_normal((D, H, HD)) * w).astype(np.float32)
    wv = (rng.standard_normal((D, H, HD)) * w).astype(np.float32)
    wo = (rng.standard_normal((H, HD, D)) / np.sqrt(FH)).astype(np.float32)
    zq = (rng.standard_normal((H, HD)) * 0.1).astype(np.float32)
    zo = (rng.standard_normal((D,)) * 0.1).astype(np.float32)

    nc = build_module()
    in_maps = shard_inputs(u, e, bpp, uem, eum, wq, zq, wk, zq, wv, zq,
                           wo, zo, np.float32(1.3), np.float32(-0.2))

    core = int(__import__("os").environ.get("CORE", "0"))
    sim = CoreSim(nc, trace=False)
    for k, vv in in_maps[core].items():
        sim.tensor(k)[:] = vv
    sim.simulate(check_with_hw=False)
    got = np.array(sim.tensor("out"))

    def ref_unit(encQ, encK, bias_qk, mask_qk):
        q = SCALE * (encQ @ wq.reshape(D, FH) + zq.reshape(FH))
        kk = encK @ wk.reshape(D, FH) + zq.reshape(FH)
        vv = encK @ wv.reshape(D, FH) + zq.reshape(FH)
        accum = np.zeros((L, D), np.float64)
        for h in range(H):
            qi = q[:, h * HD:(h + 1) * HD]
            ki = kk[:, h * HD:(h + 1) * HD]
            vi = vv[:, h * HD:(h + 1) * HD]
            s = qi @ ki.T + bias_qk
            s = np.where(mask_qk, s, -np.inf)
            s = s - s.max(-1, keepdims=True)
            p_ = np.exp(s)
            p_ /= p_.sum(-1, keepdims=True)
            accum += (p_ @ vi) @ wo[h]
        return (accum + zo).astype(np.float32)

    bq = 1.3 * bpp + -0.2
    if core < B:
        exp_out = ref_unit(u[core], e[core], bq, uem[core, 0])
    else:
        exp_out = ref_unit(e[core - B], u[core - B], bq.T, eum[core - B, 0])
    err = np.abs(got - exp_out).max() / np.abs(exp_out).max()
    print("unit relerr vs numpy:", err)
